# revision 8
# baseline (speedup 1.0000x reference)
"""Trainium2 Bass kernel for nn_CoordinateDescent (B=2, M=N=2048, R=16).

Math: the coordinate-descent residual e never needs materializing. With
G = v^T v and c = x @ v, the per-rank recurrence collapses to a 16x16
upper-triangular solve per row:  a @ L = y,  L = (D+eps) + strict_upper(G),
y = c + eps - u @ strict_lower(G).  Transposed:  aT = (I+Z)^-1 (rd .* yT)
with Z = rd .* strict_lower(G) strictly-lower-triangular (nilpotent), so
(I+Z)^-1 = (I-Z)(I+Z^2)(I+Z^4)(I+Z^8) exactly -- log-depth small matmuls,
no sequential scalar loop.

Sharding: 8 cores = batch (2) x M-shard (4). Phase 1 (u-update) needs x^T
M-shards; phase 2 (v-update) needs full u_new (bf16 AllGather within each
4-core group) and natural-layout x N-shards. Host pre-packs both layouts
(bf16) so no device transposes of x are needed. Heavy matmuls run in bf16
with f32 PSUM accumulation; Gram/recurrence math runs f32 (PE ops as
float32r single-pass). Per-shard partial Gram matrices of u_new ride the
AllGather payload so the post-collective critical path starts on the
triangular-inverse chain immediately.
"""

import os
import numpy as np
import ml_dtypes

import concourse.bass as bass
import concourse.mybir as mybir
import concourse.tile as tile
from concourse import bacc
from concourse.bass_utils import run_bass_kernel_spmd
from concourse.masks import make_identity
from concourse.tile_rust import add_dep_helper

B, M, N, R = 2, 2048, 2048, 16
NCORES = 8
NS = 4            # M-shards per batch
SH = M // NS      # 512
P = 128
KO = M // P       # 16 k-tiles of 128
TS = SH // P      # 4 sub-tiles per shard
EPS = 1e-8
AGF = NS * R + R  # 80: 64 cols of A + 16 cols of G' partial

F32 = mybir.dt.float32
F32R = mybir.dt.float32r
BF16 = mybir.dt.bfloat16
ALU = mybir.AluOpType

_CACHE = {}


USE_F32R = os.environ.get("CD_NO_F32R", "") == ""


def _r(ap):
    """View an f32 AP as float32r so the PE runs single-pass matmuls."""
    return ap.bitcast(F32R) if USE_F32R else ap


def _build_nc():
    nc = bacc.Bacc(
        "TRN2",
        target_bir_lowering=False,
        debug=False,
        num_devices=NCORES,
    )

    xt_d = nc.dram_tensor("xt", [P, KO, SH], BF16, kind="ExternalInput")   # x[b,mS,:]^T tiled
    xn_d = nc.dram_tensor("xn", [P, KO, SH], BF16, kind="ExternalInput")   # x[b,:,nS] tiled
    vf_d = nc.dram_tensor("vf", [P, KO, R], F32, kind="ExternalInput")     # v[b] tiled f32
    vb_d = nc.dram_tensor("vb", [P, KO, R], BF16, kind="ExternalInput")    # v[b] tiled bf16
    ut_d = nc.dram_tensor("ut", [R, SH], BF16, kind="ExternalInput")       # u[b,mS,:]^T
    vt_d = nc.dram_tensor("vt", [R, SH], BF16, kind="ExternalInput")       # v[b,nS,:]^T
    ou_d = nc.dram_tensor("ou", [P, TS, R], F32, kind="ExternalOutput")    # u_new shard
    ov_d = nc.dram_tensor("ov", [P, TS, R], F32, kind="ExternalOutput")    # v_new shard

    RG = [[0, 1, 2, 3], [4, 5, 6, 7]]

    with tile.TileContext(nc, num_cores=NCORES) as tc:
        with (
            tc.tile_pool(name="big", bufs=1) as big,
            tc.tile_pool(name="cst", bufs=1) as cst,
            tc.tile_pool(name="ya", bufs=1) as ya,
            tc.tile_pool(name="gps", bufs=1, space="PSUM") as gps,
            tc.tile_pool(name="sps", bufs=2, space="PSUM") as sps,
            tc.tile_pool(name="bps", bufs=2, space="PSUM") as bps,
            tc.tile_pool(name="aps", bufs=2, space="PSUM") as aps,
            tc.tile_pool(name="dram", bufs=1, space="DRAM") as dram,
        ):
            # ---------------- constant / staging tiles ----------------
            vf = cst.tile([P, KO, R], F32, tag="vf")
            vb = cst.tile([P, KO, R], BF16, tag="vb")
            ut = cst.tile([P, SH], BF16, tag="ut")      # rows 16+ zero
            vt = cst.tile([P, SH], BF16, tag="vt")      # rows 16+ zero
            ident = cst.tile([P, R], F32, tag="ident")  # I16 in rows 0:16
            misc = cst.tile([P, 8], F32, tag="misc")    # d / rd columns
            af = cst.tile([P, NS, AGF], BF16, tag="af")  # allgathered payload
            anat = cst.tile([P, TS, R], F32, tag="anat")
            ab16 = cst.tile([P, TS, R], BF16, tag="ab16")
            gpb = cst.tile([P, R], BF16, tag="gpb")     # G' partial, bf16, zero-padded
            vnat = cst.tile([P, TS, R], F32, tag="vnat")

            NSLOT = 18
            arena = cst.tile([P, 2 * NSLOT, R], F32, tag="arena")
            sun16 = cst.tile([P, 2, R], BF16, tag="sun16")

            nc.any.memzero(ut[:])
            nc.any.memzero(gpb[:])
            nc.any.memzero(vt[:])
            nc.any.memzero(ident[:])
            nc.any.memzero(arena[:])
            nc.any.memzero(sun16[:])
            make_identity(nc, ident[0:R, 0:R], nomemset=True)

            nc.sync.dma_start(vf[:], vf_d[:])
            nc.sync.dma_start(vb[:], vb_d[:])
            nc.sync.dma_start(ut[0:R, :], ut_d[:])
            nc.sync.dma_start(vt[0:R, :], vt_d[:])

            # big x streams: xt chunks first (phase 1); xn chunks gated on the
            # matching xt chunk so xn never steals phase-1 DMA bandwidth.
            CH = 4  # ko-tiles per DMA chunk (528KB each)
            xt = big.tile([P, KO, SH], BF16, tag="xt")
            xn = big.tile([P, KO, SH], BF16, tag="xn")
            xt_dmas = []
            for q in range(KO // CH):
                s = slice(q * CH, (q + 1) * CH)
                xt_dmas.append(nc.sync.dma_start(xt[:, s, :], xt_d[:, s, :]))
            for q in range(KO // CH):
                s = slice(q * CH, (q + 1) * CH)
                d = nc.scalar.dma_start(xn[:, s, :], xn_d[:, s, :])
                add_dep_helper(d.ins, xt_dmas[q].ins, sync=True,
                               reason="xn stream yields DMA bandwidth to xt")

            # ---------------- helpers ----------------
            def slot(ph, i):
                return arena[:, ph * NSLOT + i, :]          # [128, 16] zero-padded

            def slot16(ph, i):
                return arena[0:R, ph * NSLOT + i, :]        # [16, 16] live region

            def smm(out_slot16, lhsT_pad, rhs_pad):
                ps = sps.tile([R, R], F32, tag="sps")
                nc.tensor.matmul(ps[:], _r(lhsT_pad), _r(rhs_pad))
                nc.any.tensor_copy(out=out_slot16, in_=ps[:])

            I16 = ident[0:R, 0:R]

            def small_chain(ph, g_psum):
                """From G -> rd [16,1], WzT slot, strict_lower(-G) bf16 tile.

                g_psum: PSUM AP holding G, or None if slot(ph, 0) already has it.
                """
                G = slot16(ph, 0)
                if g_psum is not None:
                    nc.any.tensor_copy(out=G, in_=g_psum[:])
                d = misc[0:R, 4 * ph + 0 : 4 * ph + 1]
                rd = misc[0:R, 4 * ph + 1 : 4 * ph + 2]
                gd = slot16(ph, 1)
                nc.vector.tensor_tensor(gd, G, I16, ALU.mult)
                nc.vector.tensor_reduce(d, gd, axis=mybir.AxisListType.X, op=ALU.add)
                nc.vector.tensor_scalar_add(d, d, float(EPS))
                nc.vector.reciprocal(rd, d)
                # strict_lower(-G) as lhsT so lhsT.T = -strict_upper(G) in the
                # Y-correction matmul. Slot 15 is scratch (o2T overwrites it).
                nc.any.tensor_scalar_mul(gd, G, -1.0)
                slnf = slot16(ph, 15)
                nc.gpsimd.affine_select(
                    out=slnf, in_=gd, compare_op=ALU.is_ge, fill=0.0,
                    base=-1, pattern=[[-1, R]], channel_multiplier=1,
                )
                nc.any.tensor_copy(out=sun16[0:R, ph, :], in_=slnf)
                SL = slot16(ph, 2)
                nc.gpsimd.affine_select(
                    out=SL, in_=G, compare_op=ALU.is_ge, fill=0.0,
                    base=-1, pattern=[[-1, R]], channel_multiplier=1,
                )
                Z = slot16(ph, 3)
                nc.vector.tensor_scalar_mul(Z, SL, rd)
                smm(slot16(ph, 4), slot(ph, 3), ident[:, 0:R])  # zt1 = Z^T
                smm(slot16(ph, 5), slot(ph, 4), slot(ph, 3))   # z2
                smm(slot16(ph, 6), slot(ph, 3), slot(ph, 4))   # zt2
                smm(slot16(ph, 7), slot(ph, 6), slot(ph, 5))   # z4
                smm(slot16(ph, 8), slot(ph, 5), slot(ph, 6))   # zt4
                smm(slot16(ph, 9), slot(ph, 8), slot(ph, 7))   # z8
                nc.vector.tensor_tensor(slot16(ph, 10), I16, slot16(ph, 4), ALU.subtract)
                nc.vector.tensor_tensor(slot16(ph, 11), I16, slot16(ph, 5), ALU.add)
                nc.vector.tensor_tensor(slot16(ph, 12), I16, slot16(ph, 8), ALU.add)
                nc.vector.tensor_tensor(slot16(ph, 13), I16, slot16(ph, 9), ALU.add)
                smm(slot16(ph, 14), slot(ph, 11), slot(ph, 10))  # P1
                smm(slot16(ph, 15), slot(ph, 12), slot(ph, 13))  # o2T
                smm(slot16(ph, 16), slot(ph, 15), slot(ph, 14))  # WzT
                return rd

            def heavy(ph, lhsb, rhs_big, corr_rhs_pad, g_src, out_f32, out_b16):
                """cT accumulation + recurrence solve + A/V tile production.

                The small chain is emitted after the first 4 cT matmuls: PE is
                in-order, so the chain's PE ops run while later cT k-tiles are
                still waiting on their DMA chunks.
                """
                ct = bps.tile([R, SH], F32, tag="bps")
                for ko in range(CH):
                    nc.tensor.matmul(
                        ct[:], lhsb(ko), rhs_big(ko), start=(ko == 0), stop=False
                    )
                rd = small_chain(ph, g_src)
                for ko in range(CH, KO):
                    nc.tensor.matmul(
                        ct[:], lhsb(ko), rhs_big(ko), start=False, stop=False
                    )
                nc.tensor.matmul(
                    ct[:], sun16[:, ph, :], corr_rhs_pad, start=False, stop=True
                )
                yt = ya.tile([P, SH], F32, tag=f"yt{ph}")
                nc.any.memzero(yt[:])
                nc.vector.tensor_scalar(
                    out=yt[0:R, :], in0=ct[:], scalar1=float(EPS), scalar2=rd,
                    op0=ALU.add, op1=ALU.mult,
                )
                ap = aps.tile([P, TS * R], F32, tag="aps")
                for t in range(TS):
                    nc.tensor.matmul(
                        ap[:, t * R : (t + 1) * R],
                        _r(yt[:, t * P : (t + 1) * P]), _r(slot(ph, 16)),
                    )
                nc.any.tensor_copy(
                    out=out_f32[:].rearrange("p t r -> p (t r)"), in_=ap[:]
                )
                if out_b16 is not None:
                    nc.any.tensor_copy(
                        out=out_b16[:].rearrange("p t r -> p (t r)"), in_=ap[:]
                    )

            # ================= phase 1: u update =================
            gp = gps.tile([R, R], F32, tag="gps")
            for ko in range(KO):
                nc.tensor.matmul(
                    gp[:], _r(vf[:, ko, :]), _r(vf[:, ko, :]),
                    start=(ko == 0), stop=(ko == KO - 1),
                )
            heavy(0, lambda ko: vb[:, ko, :], lambda ko: xt[:, ko, :],
                  ut[:], gp, anat, ab16)
            nc.sync.dma_start(ou_d[:], anat[:])

            # G' partial = A_s^T A_s (f32r), shipped inside the AG payload
            gp2 = gps.tile([R, R], F32, tag="gps")
            for t in range(TS):
                nc.tensor.matmul(
                    gp2[:], _r(anat[:, t, :]), _r(anat[:, t, :]),
                    start=(t == 0), stop=(t == TS - 1),
                )
            nc.any.tensor_copy(out=gpb[0:R, :], in_=gp2[:])

            # ================= allgather u_new + G' partials =================
            ag_in = dram.tile([P, AGF], BF16, tag="ag_in")
            ag_out = dram.tile([NS * P, AGF], BF16, tag="ag_out")
            nc.sync.dma_start(
                ag_in[:, 0 : NS * R], ab16[:].rearrange("p t r -> p (t r)")
            )
            nc.sync.dma_start(ag_in[:, NS * R : AGF], gpb[:])
            nc.gpsimd.collective_compute(
                "AllGather",
                ALU.bypass,
                replica_groups=RG,
                ins=[ag_in[:].opt()],
                outs=[ag_out[:].opt()],
            )
            nc.sync.dma_start(
                af[:], ag_out[:].rearrange("(g p) f -> p g f", p=P)
            )

            # ================= phase 2: v update =================
            # G' = sum of the 4 partials that rode the AG
            Gp = slot16(1, 0)
            tmp = slot16(1, 17)
            nc.vector.tensor_tensor(
                Gp, af[0:R, 0, NS * R : AGF], af[0:R, 1, NS * R : AGF], ALU.add
            )
            nc.vector.tensor_tensor(
                tmp, af[0:R, 2, NS * R : AGF], af[0:R, 3, NS * R : AGF], ALU.add
            )
            nc.vector.tensor_tensor(Gp, Gp, tmp, ALU.add)
            heavy(
                1,
                lambda ko: af[:, ko // TS, (ko % TS) * R : (ko % TS + 1) * R],
                lambda ko: xn[:, ko, :],
                vt[:], None, vnat, None,
            )
            nc.sync.dma_start(ov_d[:], vnat[:])

    nc.compile()
    return nc


def _pack(a, tiles, dtype):
    a = np.ascontiguousarray(a)
    return np.ascontiguousarray(
        a.reshape(tiles, P, *a.shape[1:]).swapaxes(0, 1)
    ).astype(dtype, copy=False)


def _prep_in_maps(x, u, v):
    bf = ml_dtypes.bfloat16
    in_maps = []
    for c in range(NCORES):
        b, s = divmod(c, NS)
        sl = slice(s * SH, (s + 1) * SH)
        xb = np.asarray(x[b], np.float32)
        xt = _pack(np.ascontiguousarray(xb[sl].T).astype(bf), KO, bf)
        xn = _pack(np.ascontiguousarray(xb[:, sl]).astype(bf), KO, bf)
        vf = _pack(np.asarray(v[b], np.float32), KO, np.float32)
        vb = vf.astype(bf)
        ut = np.ascontiguousarray(np.asarray(u[b], np.float32)[sl].T).astype(bf)
        vt = np.ascontiguousarray(np.asarray(v[b], np.float32)[sl].T).astype(bf)
        in_maps.append(
            {"xt": xt, "xn": xn, "vf": vf, "vb": vb, "ut": ut, "vt": vt}
        )
    return in_maps


def run(x, u, v, trace=False, trace_cores=None):
    if "nc" not in _CACHE:
        _CACHE["nc"] = _build_nc()
    nc = _CACHE["nc"]
    in_maps = _prep_in_maps(x, u, v)
    kw = {}
    if trace_cores is not None:
        kw["trace_cores"] = trace_cores
    res = run_bass_kernel_spmd(
        nc, in_maps, core_ids=list(range(NCORES)), trace=trace, **kw
    )
    u_new = np.empty((B, M, R), np.float32)
    v_new = np.empty((B, M, R), np.float32)
    for c in range(NCORES):
        b, s = divmod(c, NS)
        sl = slice(s * SH, (s + 1) * SH)
        u_new[b, sl] = (
            np.asarray(res.results[c]["ou"]).transpose(1, 0, 2).reshape(SH, R)
        )
        v_new[b, sl] = (
            np.asarray(res.results[c]["ov"]).transpose(1, 0, 2).reshape(SH, R)
        )
    return (u_new, v_new), res


def kernel(x, u, v):
    (u_new, v_new), _ = run(x, u, v, trace=bool(os.environ.get("CD_TRACE")))
    return (u_new, v_new)
